# revision 9
# baseline (speedup 1.0000x reference)
"""Cross-attention kernel for Trainium2, SPMD across 8 NeuronCores.

Problem shapes (hardcoded): x [4, 2048, 512], mlp_out [4, 2048, 512],
Wq/Wk/Wv/Wp [512, 512], biases [512]. 8 heads x 64 head-dim.

Sharding: core c handles batch b = c//2 and query rows
[(c%2)*1024 : (c%2+1)*1024).  K/V work is duplicated across the two
cores of a batch pair; no collective is needed.

Design (v2):
  - Everything bf16 on SBUF (halves DMA + SBUF vs f32r); PSUM fp32.
  - The ACT engine's 128 exp instructions (~127us) are the hard floor:
    schedule starts exp ASAP and keeps it saturated.
  - Projections produce contraction-major tensors directly:
      qT/kT: [dh, seq] via stationary Wq^T/Wk^T column blocks,
      vaug:  [n, h, d+1] with a trailing ones column (softmax denom).
  - logits computed transposed [k, q] per (head, kt); exp -> attnT bf16.
  - AV uses vaug as the STATIONARY operand (out [d+1, q] per head),
    16 matmuls of 512 moving columns per (h, qb): few instructions,
    and the output lands head-dim-major, feeding the output projection
    without any transposes.
  - Softmax division: reciprocal of the denom row, PE-broadcast across
    64 partitions (ones[1,64] x recip-row), then one DVE
    scalar_tensor_tensor multiply evicts AV into aoT bf16.
  - Output projection emits outT [co, q]; the HOST transposes (free).
"""

import numpy as np

import concourse.bass as bass
import concourse.tile as tile
from concourse import bacc, mybir
from concourse.bass_utils import run_bass_kernel_spmd

B = 4
N = 2048          # key/value sequence length
C = 512           # model dim
H = 8
D = C // H        # 64
NCORES = 8
QSH = N // 2      # query rows per core (1024)

F32 = mybir.dt.float32
F32R = mybir.dt.float32r
BF16 = mybir.dt.bfloat16

P = 128
CT = C // P       # 4 tiles along any model-dim axis
KT = N // P       # 16 key tiles
QB = QSH // 512   # 2 query 512-blocks
NB = N // 512     # 4 key 512-blocks
DHT = CT          # 4 dh tiles (2 heads each)

MULT = mybir.AluOpType.mult


def build_nc(with_bias: bool, reps: int = 1):
    nc = bacc.Bacc("TRN2", target_bir_lowering=False, debug=False)

    xT = nc.dram_tensor("xT", [C, QSH], BF16, kind="ExternalInput")
    mlpT = nc.dram_tensor("mlpT", [C, N], BF16, kind="ExternalInput")
    wqT = nc.dram_tensor("wqT", [C, C], BF16, kind="ExternalInput")
    wkT = nc.dram_tensor("wkT", [C, C], BF16, kind="ExternalInput")
    wvT = nc.dram_tensor("wvT", [C, C], BF16, kind="ExternalInput")
    wpT = nc.dram_tensor("wpT", [C, C], BF16, kind="ExternalInput")
    if with_bias:
        bqc = nc.dram_tensor("bqc", [C, 1], F32, kind="ExternalInput")
        bkc = nc.dram_tensor("bkc", [C, 1], F32, kind="ExternalInput")
        bvr = nc.dram_tensor("bvr", [1, C], F32, kind="ExternalInput")
        bpc = nc.dram_tensor("bpc", [C, 1], F32, kind="ExternalInput")
    out = nc.dram_tensor("out", [C, QSH], F32, kind="ExternalOutput")

    with tile.TileContext(nc) as tc:
        from contextlib import ExitStack

        with ExitStack() as ctx:
            const = ctx.enter_context(tc.tile_pool(name="const", bufs=1))
            w_pool = ctx.enter_context(tc.tile_pool(name="w", bufs=1))
            x_pool = ctx.enter_context(tc.tile_pool(name="x", bufs=1))
            m_pool = ctx.enter_context(tc.tile_pool(name="mlp", bufs=1))
            qt_pool = ctx.enter_context(tc.tile_pool(name="qT", bufs=1))
            kt_pool = ctx.enter_context(tc.tile_pool(name="kT", bufs=1))
            v_pool = ctx.enter_context(tc.tile_pool(name="vaug", bufs=1))
            attn_pool = ctx.enter_context(tc.tile_pool(name="attnT", bufs=46))
            ao_pool = ctx.enter_context(tc.tile_pool(name="aoT", bufs=1))
            outst = ctx.enter_context(tc.tile_pool(name="outst", bufs=2))
            small = ctx.enter_context(tc.tile_pool(name="small", bufs=2))
            lp_ps = ctx.enter_context(
                tc.tile_pool(name="lp_ps", bufs=2, space="PSUM")
            )
            av_ps = ctx.enter_context(
                tc.tile_pool(name="av_ps", bufs=2, space="PSUM")
            )
            proj_ps = ctx.enter_context(
                tc.tile_pool(name="proj_ps", bufs=2, space="PSUM")
            )

            ones_f = const.tile([1, P], F32)
            nc.vector.memset(ones_f[:], 1.0)
            ones_r = const.tile([1, P], F32R)
            nc.vector.tensor_copy(ones_r[:], ones_f[:])
            ones_b = const.tile([1, P], BF16)
            nc.vector.memset(ones_b[:], 1.0)

            for _rep in range(reps):

                # ---- resident tiles ----
                wq = [w_pool.tile([P, C], BF16, tag=f"wq{i}", name=f"wq{i}")
                      for i in range(CT)]
                wk = [w_pool.tile([P, C], BF16, tag=f"wk{i}", name=f"wk{i}")
                      for i in range(CT)]
                wv = [w_pool.tile([P, C], BF16, tag=f"wv{i}", name=f"wv{i}")
                      for i in range(CT)]
                wp = [w_pool.tile([P, C], BF16, tag=f"wp{i}", name=f"wp{i}")
                      for i in range(CT)]
                xt = [x_pool.tile([P, QSH], BF16, tag=f"x{i}", name=f"x{i}")
                      for i in range(CT)]
                mt = [m_pool.tile([P, N], BF16, tag=f"m{i}", name=f"m{i}")
                      for i in range(CT)]
                qT = [qt_pool.tile([P, QSH], BF16, tag=f"qT{i}", name=f"qT{i}")
                      for i in range(DHT)]
                kT = [kt_pool.tile([P, N], BF16, tag=f"kT{i}", name=f"kT{i}")
                      for i in range(DHT)]
                vaug = [v_pool.tile([P, H, D + 1], BF16, tag=f"v{i}",
                                    name=f"v{i}") for i in range(KT)]
                aoT = [ao_pool.tile([P, QSH], BF16, tag=f"ao{i}", name=f"ao{i}")
                       for i in range(DHT)]

                # ---- DMA prologue, critical-path first ----
                def dma_w(tiles, dram):
                    for i, t in enumerate(tiles):
                        nc.sync.dma_start(out=t[:], in_=dram[i * P:(i + 1) * P, :])

                def dma_chunk(t, dram, row0, col0, cols):
                    nc.sync.dma_start(
                        out=t[:, col0:col0 + cols],
                        in_=dram[row0:row0 + P, col0:col0 + cols],
                    )

                dma_w(wq, wqT)
                for i in range(CT):          # x qb0 chunks
                    dma_chunk(xt[i], xT, i * P, 0, 512)
                dma_w(wk, wkT)
                for i in range(CT):          # mlp nb0 chunks
                    dma_chunk(mt[i], mlpT, i * P, 0, 512)
                for i in range(CT):          # x qb1
                    dma_chunk(xt[i], xT, i * P, 512, 512)
                for nb in range(1, NB):      # rest of mlp
                    for i in range(CT):
                        dma_chunk(mt[i], mlpT, i * P, nb * 512, 512)
                dma_w(wv, wvT)
                dma_w(wp, wpT)

                if with_bias:
                    bq_col = [small.tile([P, 1], F32, tag=f"bq{i}", name=f"bq{i}")
                              for i in range(DHT)]
                    bk_col = [small.tile([P, 1], F32, tag=f"bk{i}", name=f"bk{i}")
                              for i in range(DHT)]
                    bp_col = [small.tile([P, 1], F32, tag=f"bp{i}", name=f"bp{i}")
                              for i in range(CT)]
                    for i in range(DHT):
                        nc.sync.dma_start(out=bq_col[i][:],
                                          in_=bqc[i * P:(i + 1) * P, :])
                        nc.sync.dma_start(out=bk_col[i][:],
                                          in_=bkc[i * P:(i + 1) * P, :])
                        nc.sync.dma_start(out=bp_col[i][:],
                                          in_=bpc[i * P:(i + 1) * P, :])
                    bv_f = const.tile([1, C], F32, name="bv_f")
                    nc.sync.dma_start(out=bv_f[:], in_=bvr[:])
                    bv_r = const.tile([1, C], F32R, name="bv_r")
                    nc.vector.tensor_copy(bv_r[:], bv_f[:])

                # ---- work chunks ----
                def proj_qk(dh, wt, dst, seqlen, src, nb_, bias_col=None):
                    # dst[dh][:, nb*512:...] = (wt col-block dh).T @ src
                    ps = proj_ps.tile([P, 512], F32, tag="proj", name="ps_qk")
                    for cb in range(CT):
                        nc.tensor.matmul(
                            ps[:],
                            wt[cb][:, dh * P:(dh + 1) * P],
                            src[cb][:, nb_ * 512:(nb_ + 1) * 512],
                            start=(cb == 0),
                            stop=(cb == CT - 1),
                        )
                    sl = dst[dh][:, nb_ * 512:(nb_ + 1) * 512]
                    if bias_col is not None:
                        nc.vector.tensor_scalar_add(sl, ps[:], bias_col[dh][:])
                    else:
                        nc.vector.tensor_copy(sl, ps[:])

                def proj_v(nt):
                    # vaug[nt][:, :, 0:D] = mlp rows-block nt @ Wv^T
                    nb_ = nt // 4
                    lo = (nt % 4) * P
                    ps = proj_ps.tile([P, 512], F32, tag="proj", name="ps_v")
                    for cb in range(CT):
                        nc.tensor.matmul(
                            ps[:],
                            mt[cb][:, nt * P:(nt + 1) * P],
                            wv[cb][:],
                            start=(cb == 0),
                            stop=(cb == CT - 1 and not with_bias),
                        )
                    if with_bias:
                        nc.tensor.matmul(ps[:], ones_r[:], bv_r[:],
                                         start=False, stop=True)
                    nc.vector.tensor_copy(
                        vaug[nt][:, :, 0:D],
                        ps[:].rearrange("p (h d) -> p h d", h=H),
                    )
                    nc.vector.memset(vaug[nt][:, :, D:D + 1], 1.0)

                attnT = {h: [] for h in range(H)}

                def logits_exp(h, kt):
                    dh, po = h // 2, (h % 2) * D
                    lp = lp_ps.tile([P, QSH], F32, tag="lp", name="lp")
                    for qb in range(QB):
                        nc.tensor.matmul(
                            lp[:, qb * 512:(qb + 1) * 512],
                            kT[dh][po:po + D, kt * P:(kt + 1) * P],
                            qT[dh][po:po + D, qb * 512:(qb + 1) * 512],
                            start=True,
                            stop=True,
                        )
                    at = attn_pool.tile([P, QSH], BF16, tag="attnT", name="at")
                    nc.scalar.activation(
                        out=at[:], in_=lp[:],
                        func=mybir.ActivationFunctionType.Exp,
                    )
                    attnT[h].append(at)

                def av_alloc():
                    return [av_ps.tile([P, 512], F32, tag="av", name="av")
                            for _ in range(QB)]

                def av_mm(h, kt, av):
                    tiles = attnT[h]
                    for qb in range(QB):
                        nc.tensor.matmul(
                            av[qb][0:D + 1, :],
                            vaug[kt][:, h, :],
                            tiles[kt][:, qb * 512:(qb + 1) * 512],
                            start=(kt == 0),
                            stop=(kt == KT - 1),
                        )

                def av_fin(h, av):
                    dh, po = h // 2, (h % 2) * D
                    for qb in range(QB):
                        rdn = small.tile([1, 512], F32, tag="rdn", name="rdn")
                        nc.vector.reciprocal(rdn[:], av[qb][D:D + 1, :])
                        rdnr = small.tile([1, 512], BF16, tag="rdnr", name="rdnr")
                        nc.vector.tensor_copy(rdnr[:], rdn[:])
                        nc.tensor.matmul(
                            av[qb][64:128, :], ones_b[:, 0:64], rdnr[:],
                            start=True, stop=True,
                        )
                        # TensorScalarPtr may read only one PSUM operand:
                        # bounce the broadcast recip through SBUF.
                        rbc = small.tile([D, 512], F32, tag="rbc", name="rbc")
                        nc.vector.tensor_copy(rbc[:], av[qb][64:128, :])
                        nc.vector.scalar_tensor_tensor(
                            out=aoT[dh][po:po + D, qb * 512:(qb + 1) * 512],
                            in0=av[qb][0:D, :],
                            scalar=1.0,
                            in1=rbc[:],
                            op0=MULT,
                            op1=MULT,
                        )

                def out_proj(qb, co):
                    ps = proj_ps.tile([P, 512], F32, tag="proj", name="ps_o")
                    for cb in range(CT):
                        nc.tensor.matmul(
                            ps[:],
                            wp[cb][:, co * P:(co + 1) * P],
                            aoT[cb][:, qb * 512:(qb + 1) * 512],
                            start=(cb == 0),
                            stop=(cb == CT - 1),
                        )
                    o = outst.tile([P, 512], F32, tag="outst", name="outst")
                    if with_bias:
                        nc.vector.tensor_scalar_add(o[:], ps[:], bp_col[co][:])
                    else:
                        nc.vector.tensor_copy(o[:], ps[:])
                    nc.sync.dma_start(
                        out=out[co * P:(co + 1) * P, qb * 512:(qb + 1) * 512],
                        in_=o[:],
                    )

                # ---- schedule ----
                bq_arg = bq_col if with_bias else None
                bk_arg = bk_col if with_bias else None

                def Qp(dh, qb):
                    proj_qk(dh, wq, qT, QSH, xt, qb, bq_arg)

                def Kp(dh, nb):
                    proj_qk(dh, wk, kT, N, mt, nb, bk_arg)

                # prologue: qT/kT for heads 0/1, logits+exp head 0
                for qb in range(QB):
                    Qp(0, qb)
                for nb in range(NB):
                    Kp(0, nb)
                    for kt in range(4 * nb, 4 * nb + 4):
                        logits_exp(0, kt)

                # Slot s is emitted while ACT chews exp(head s).  Each slot
                # feeds logits for head s+1 just-in-time, runs one lagged AV,
                # and drains a deadline-ordered queue of projection chunks
                # (qT/kT for later heads, V for AV).  Deadlines: kT[dh]
                # before logits(2dh) chunks; all vaug before av(0) in slot 2.
                extra = {
                    0: [lambda qb=qb: Qp(1, qb) for qb in range(QB)]
                    + [lambda nb=nb: Kp(1, nb) for nb in range(NB)],
                    1: [lambda nt=nt: proj_v(nt) for nt in range(0, 12)]
                    + [lambda qb=qb: Qp(2, qb) for qb in range(QB)],
                    2: [lambda nt=nt: proj_v(nt) for nt in range(12, 16)]
                    + [lambda nb=nb: Kp(2, nb) for nb in range(NB)],
                    3: [lambda qb=qb: Qp(3, qb) for qb in range(QB)]
                    + [lambda nb=nb: Kp(3, nb) for nb in range(NB)],
                    4: [], 5: [], 6: [], 7: [],
                }
                # av(h): (head, kt_lo, kt_hi) emission ranges per slot.
                av_sched = {
                    0: [], 1: [],
                    2: [(0, 0, KT)],
                    3: [(1, 0, KT)],
                    4: [(2, 0, KT)],
                    5: [(3, 0, KT)],
                    6: [(4, 0, KT), (5, 0, 8)],
                    7: [(5, 8, KT), (6, 0, KT), (7, 0, 8)],
                }
                av_live: dict[int, list] = {}

                def av_step(h, kt):
                    if h not in av_live:
                        av_live[h] = av_alloc()
                    av_mm(h, kt, av_live[h])
                    if kt == KT - 1:
                        av_fin(h, av_live.pop(h))

                for slot in range(H):
                    nxt = slot + 1
                    work = list(extra[slot])
                    avq = [(h, kt) for (h, lo, hi) in av_sched[slot]
                           for kt in range(lo, hi)]
                    # spread av steps evenly across the 8 logits pairs
                    per_i = (len(avq) + 7) // 8
                    for i in range(8):
                        if nxt < H:
                            logits_exp(nxt, 2 * i)
                            logits_exp(nxt, 2 * i + 1)
                        if work:
                            work.pop(0)()
                        for _ in range(per_i):
                            if avq:
                                av_step(*avq.pop(0))
                    while work:
                        work.pop(0)()
                    while avq:
                        av_step(*avq.pop(0))

                # tail: finish av(7), then output projection
                for kt in range(8, KT):
                    av_step(7, kt)
                for qb in range(QB):
                    for co in range(CT):
                        out_proj(qb, co)

    nc.compile()
    return nc


_CACHE: dict = {}


def get_nc(with_bias: bool):
    key = ("nc", with_bias)
    if key not in _CACHE:
        _CACHE[key] = build_nc(with_bias)
    return _CACHE[key]


def _bf16(a):
    import ml_dtypes
    return np.ascontiguousarray(a.astype(ml_dtypes.bfloat16))


def make_in_maps(inputs: dict) -> tuple[list[dict], bool]:
    x = np.asarray(inputs["x"], dtype=np.float32)
    mlp = np.asarray(inputs["mlp_out"], dtype=np.float32)
    Wq = np.asarray(inputs["Wq"], dtype=np.float32)
    Wk = np.asarray(inputs["Wk"], dtype=np.float32)
    Wv = np.asarray(inputs["Wv"], dtype=np.float32)
    Wp = np.asarray(inputs["Wp"], dtype=np.float32)
    bq = np.asarray(inputs["bq"], dtype=np.float32)
    bk = np.asarray(inputs["bk"], dtype=np.float32)
    bv = np.asarray(inputs["bv"], dtype=np.float32)
    bp = np.asarray(inputs["bp"], dtype=np.float32)

    with_bias = bool(np.any(bq) or np.any(bk) or np.any(bv) or np.any(bp))

    wqT = _bf16(Wq.T)
    wkT = _bf16(Wk.T)
    wvT = _bf16(Wv.T)
    wpT = _bf16(Wp.T)

    in_maps = []
    for c in range(NCORES):
        b, half = c // 2, c % 2
        xs = _bf16(x[b, half * QSH:(half + 1) * QSH, :].T)
        ms = _bf16(mlp[b].T)
        m = {
            "xT": xs, "mlpT": ms,
            "wqT": wqT, "wkT": wkT, "wvT": wvT, "wpT": wpT,
        }
        if with_bias:
            m["bqc"] = bq.reshape(C, 1)
            m["bkc"] = bk.reshape(C, 1)
            m["bvr"] = bv.reshape(1, C)
            m["bpc"] = bp.reshape(C, 1)
        in_maps.append(m)
    return in_maps, with_bias


def kernel(**inputs) -> np.ndarray:
    in_maps, with_bias = make_in_maps(inputs)
    nc = get_nc(with_bias)
    res = run_bass_kernel_spmd(nc, in_maps, list(range(NCORES)))
    full = np.empty((B, N, C), dtype=np.float32)
    for c in range(NCORES):
        b, half = c // 2, c % 2
        full[b, half * QSH:(half + 1) * QSH, :] = res.results[c]["out"].T
    return full


# revision 14
# speedup vs baseline: 1.0896x; 1.0896x over previous
"""Cross-attention kernel for Trainium2, SPMD across 8 NeuronCores.

Problem shapes (hardcoded): x [4, 2048, 512], mlp_out [4, 2048, 512],
Wq/Wk/Wv/Wp [512, 512], biases [512]. 8 heads x 64 head-dim.

Sharding: core c handles batch b = c//2 and query rows
[(c%2)*1024 : (c%2+1)*1024).  K/V work is duplicated across the two
cores of a batch pair; no collective is needed.

Design (v2):
  - Everything bf16 on SBUF (halves DMA + SBUF vs f32r); PSUM fp32.
  - The ACT engine's 128 exp instructions (~127us) are the hard floor:
    schedule starts exp ASAP and keeps it saturated.
  - Projections produce contraction-major tensors directly:
      qT/kT: [dh, seq] via stationary Wq^T/Wk^T column blocks,
      vaug:  [n, h, d+1] with a trailing ones column (softmax denom).
  - logits computed transposed [k, q] per (head, kt); exp -> attnT bf16.
  - AV uses vaug as the STATIONARY operand (out [d+1, q] per head),
    16 matmuls of 512 moving columns per (h, qb): few instructions,
    and the output lands head-dim-major, feeding the output projection
    without any transposes.
  - Softmax division: reciprocal of the denom row, PE-broadcast across
    64 partitions (ones[1,64] x recip-row), then one DVE
    scalar_tensor_tensor multiply evicts AV into aoT bf16.
  - Output projection emits outT [co, q]; the HOST transposes (free).
"""

import numpy as np

import concourse.bass as bass
import concourse.tile as tile
from concourse import bacc, mybir
from concourse.bass_utils import run_bass_kernel_spmd

B = 4
N = 2048          # key/value sequence length
C = 512           # model dim
H = 8
D = C // H        # 64
NCORES = 8
QSH = N // 2      # query rows per core (1024)

F32 = mybir.dt.float32
F32R = mybir.dt.float32r
BF16 = mybir.dt.bfloat16

P = 128
CT = C // P       # 4 tiles along any model-dim axis
KT = N // P       # 16 key tiles
QB = QSH // 512   # 2 query 512-blocks
NB = N // 512     # 4 key 512-blocks
DHT = CT          # 4 dh tiles (2 heads each)

MULT = mybir.AluOpType.mult


def build_nc(with_bias: bool, reps: int = 1):
    nc = bacc.Bacc("TRN2", target_bir_lowering=False, debug=False)

    xT = nc.dram_tensor("xT", [C, QSH], BF16, kind="ExternalInput")
    mlpT = nc.dram_tensor("mlpT", [C, N], BF16, kind="ExternalInput")
    wqT = nc.dram_tensor("wqT", [C, C], BF16, kind="ExternalInput")
    wkT = nc.dram_tensor("wkT", [C, C], BF16, kind="ExternalInput")
    wvT = nc.dram_tensor("wvT", [C, C], BF16, kind="ExternalInput")
    wpT = nc.dram_tensor("wpT", [C, C], BF16, kind="ExternalInput")
    if with_bias:
        bqc = nc.dram_tensor("bqc", [C, 1], F32, kind="ExternalInput")
        bkc = nc.dram_tensor("bkc", [C, 1], F32, kind="ExternalInput")
        bvr = nc.dram_tensor("bvr", [1, C], F32, kind="ExternalInput")
        bpc = nc.dram_tensor("bpc", [C, 1], F32, kind="ExternalInput")
    out = nc.dram_tensor("out", [C, QSH], F32, kind="ExternalOutput")

    with tile.TileContext(nc) as tc:
        from contextlib import ExitStack

        with ExitStack() as ctx:
            const = ctx.enter_context(tc.tile_pool(name="const", bufs=1))
            w_pool = ctx.enter_context(tc.tile_pool(name="w", bufs=1))
            x_pool = ctx.enter_context(tc.tile_pool(name="x", bufs=1))
            m_pool = ctx.enter_context(tc.tile_pool(name="mlp", bufs=1))
            qt_pool = ctx.enter_context(tc.tile_pool(name="qT", bufs=1))
            kt_pool = ctx.enter_context(tc.tile_pool(name="kT", bufs=1))
            v_pool = ctx.enter_context(tc.tile_pool(name="vaug", bufs=1))
            attn_pool = ctx.enter_context(tc.tile_pool(name="attnT", bufs=40))
            ao_pool = ctx.enter_context(tc.tile_pool(name="aoT", bufs=1))
            outst = ctx.enter_context(tc.tile_pool(name="outst", bufs=2))
            small = ctx.enter_context(tc.tile_pool(name="small", bufs=2))
            lp_ps = ctx.enter_context(
                tc.tile_pool(name="lp_ps", bufs=2, space="PSUM")
            )
            av_ps = ctx.enter_context(
                tc.tile_pool(name="av_ps", bufs=2, space="PSUM")
            )
            proj_ps = ctx.enter_context(
                tc.tile_pool(name="proj_ps", bufs=2, space="PSUM")
            )

            ones_f = const.tile([1, P], F32)
            nc.vector.memset(ones_f[:], 1.0)
            ones_r = const.tile([1, P], F32R)
            nc.vector.tensor_copy(ones_r[:], ones_f[:])
            ones_b = const.tile([1, P], BF16)
            nc.vector.memset(ones_b[:], 1.0)

            for _rep in range(reps):

                # ---- resident tiles ----
                wq = [w_pool.tile([P, C], BF16, tag=f"wq{i}", name=f"wq{i}")
                      for i in range(CT)]
                wk = [w_pool.tile([P, C], BF16, tag=f"wk{i}", name=f"wk{i}")
                      for i in range(CT)]
                wv = [w_pool.tile([P, C], BF16, tag=f"wv{i}", name=f"wv{i}")
                      for i in range(CT)]
                wp = [w_pool.tile([P, C], BF16, tag=f"wp{i}", name=f"wp{i}")
                      for i in range(CT)]
                xt = [x_pool.tile([P, QSH], BF16, tag=f"x{i}", name=f"x{i}")
                      for i in range(CT)]
                mt = [m_pool.tile([P, N], BF16, tag=f"m{i}", name=f"m{i}")
                      for i in range(CT)]
                # Per-head query tiles with the sibling head's 64 partition
                # rows held at zero: lets the logits matmul use the full
                # two-head kT tile as a K=128 stationary (the K=64 matmul
                # shape measures ~7x slower per instruction on HW).
                qTz = [qt_pool.tile([P, QSH], BF16, tag=f"qz{h}", name=f"qz{h}")
                       for h in range(H)]
                kT = [kt_pool.tile([P, N], BF16, tag=f"kT{i}", name=f"kT{i}")
                      for i in range(DHT)]
                vaug = [v_pool.tile([P, H, D + 1], BF16, tag=f"v{i}",
                                    name=f"v{i}") for i in range(KT)]
                aoT = [ao_pool.tile([P, QSH], BF16, tag=f"ao{i}", name=f"ao{i}")
                       for i in range(DHT)]

                # ---- DMA prologue, critical-path first ----
                def dma_w(tiles, dram):
                    for i, t in enumerate(tiles):
                        nc.sync.dma_start(out=t[:], in_=dram[i * P:(i + 1) * P, :])

                def dma_chunk(t, dram, row0, col0, cols):
                    nc.sync.dma_start(
                        out=t[:, col0:col0 + cols],
                        in_=dram[row0:row0 + P, col0:col0 + cols],
                    )

                dma_w(wq, wqT)
                for i in range(CT):          # x qb0 chunks
                    dma_chunk(xt[i], xT, i * P, 0, 512)
                dma_w(wk, wkT)
                for i in range(CT):          # mlp nb0 chunks
                    dma_chunk(mt[i], mlpT, i * P, 0, 512)
                for i in range(CT):          # x qb1
                    dma_chunk(xt[i], xT, i * P, 512, 512)
                for nb in range(1, NB):      # rest of mlp
                    for i in range(CT):
                        dma_chunk(mt[i], mlpT, i * P, nb * 512, 512)
                dma_w(wv, wvT)
                dma_w(wp, wpT)

                if with_bias:
                    bq_col = [small.tile([P, 1], F32, tag=f"bq{i}", name=f"bq{i}")
                              for i in range(DHT)]
                    bk_col = [small.tile([P, 1], F32, tag=f"bk{i}", name=f"bk{i}")
                              for i in range(DHT)]
                    bp_col = [small.tile([P, 1], F32, tag=f"bp{i}", name=f"bp{i}")
                              for i in range(CT)]
                    for i in range(DHT):
                        nc.sync.dma_start(out=bq_col[i][:],
                                          in_=bqc[i * P:(i + 1) * P, :])
                        nc.sync.dma_start(out=bk_col[i][:],
                                          in_=bkc[i * P:(i + 1) * P, :])
                        nc.sync.dma_start(out=bp_col[i][:],
                                          in_=bpc[i * P:(i + 1) * P, :])
                    bv_f = const.tile([1, C], F32, name="bv_f")
                    nc.sync.dma_start(out=bv_f[:], in_=bvr[:])
                    bv_r = const.tile([1, C], F32R, name="bv_r")
                    nc.vector.tensor_copy(bv_r[:], bv_f[:])

                # ---- work chunks ----
                def _proj_mm(wt, dh, src, nb_):
                    ps = proj_ps.tile([P, 512], F32, tag="proj", name="ps_qk")
                    for cb in range(CT):
                        nc.tensor.matmul(
                            ps[:],
                            wt[cb][:, dh * P:(dh + 1) * P],
                            src[cb][:, nb_ * 512:(nb_ + 1) * 512],
                            start=(cb == 0),
                            stop=(cb == CT - 1),
                        )
                    return ps

                def proj_q(dh, qb, bias_col=None):
                    # evict each head's 64 rows into its zero-padded tile
                    ps = _proj_mm(wq, dh, xt, qb)
                    for par in range(2):
                        h = 2 * dh + par
                        po = par * D
                        sl = qTz[h][po:po + D, qb * 512:(qb + 1) * 512]
                        if bias_col is not None:
                            nc.vector.tensor_scalar_add(
                                sl, ps[po:po + D, :], bias_col[dh][po:po + D, :])
                        else:
                            nc.vector.tensor_copy(sl, ps[po:po + D, :])

                def proj_k(dh, nb_, bias_col=None):
                    ps = _proj_mm(wk, dh, mt, nb_)
                    sl = kT[dh][:, nb_ * 512:(nb_ + 1) * 512]
                    if bias_col is not None:
                        nc.vector.tensor_scalar_add(sl, ps[:], bias_col[dh][:])
                    else:
                        nc.vector.tensor_copy(sl, ps[:])

                def proj_v(nt):
                    # vaug[nt][:, :, 0:D] = mlp rows-block nt @ Wv^T
                    nb_ = nt // 4
                    lo = (nt % 4) * P
                    ps = proj_ps.tile([P, 512], F32, tag="proj", name="ps_v")
                    for cb in range(CT):
                        nc.tensor.matmul(
                            ps[:],
                            mt[cb][:, nt * P:(nt + 1) * P],
                            wv[cb][:],
                            start=(cb == 0),
                            stop=(cb == CT - 1 and not with_bias),
                        )
                    if with_bias:
                        nc.tensor.matmul(ps[:], ones_r[:], bv_r[:],
                                         start=False, stop=True)
                    nc.vector.tensor_copy(
                        vaug[nt][:, :, 0:D],
                        ps[:].rearrange("p (h d) -> p h d", h=H),
                    )
                    nc.vector.memset(vaug[nt][:, :, D:D + 1], 1.0)

                attnT = {h: [] for h in range(H)}

                def logits_exp(h, kt):
                    dh = h // 2
                    lp = lp_ps.tile([P, QSH], F32, tag="lp", name="lp")
                    for qb in range(QB):
                        nc.tensor.matmul(
                            lp[:, qb * 512:(qb + 1) * 512],
                            kT[dh][:, kt * P:(kt + 1) * P],
                            qTz[h][:, qb * 512:(qb + 1) * 512],
                            start=True,
                            stop=True,
                        )
                    at = attn_pool.tile([P, QSH], BF16, tag="attnT", name="at")
                    nc.scalar.activation(
                        out=at[:], in_=lp[:],
                        func=mybir.ActivationFunctionType.Exp,
                    )
                    attnT[h].append(at)

                def av_alloc():
                    return [av_ps.tile([P, 512], F32, tag="av", name="av")
                            for _ in range(QB)]

                def av_mm(h, kt, av):
                    tiles = attnT[h]
                    for qb in range(QB):
                        nc.tensor.matmul(
                            av[qb][0:D + 1, :],
                            vaug[kt][:, h, :],
                            tiles[kt][:, qb * 512:(qb + 1) * 512],
                            start=(kt == 0),
                            stop=(kt == KT - 1),
                        )

                def av_fin(h, av):
                    dh, po = h // 2, (h % 2) * D
                    for qb in range(QB):
                        rdn = small.tile([1, 512], F32, tag="rdn", name="rdn")
                        nc.vector.reciprocal(rdn[:], av[qb][D:D + 1, :])
                        rdnr = small.tile([1, 512], BF16, tag="rdnr", name="rdnr")
                        nc.vector.tensor_copy(rdnr[:], rdn[:])
                        nc.tensor.matmul(
                            av[qb][64:128, :], ones_b[:, 0:64], rdnr[:],
                            start=True, stop=True,
                        )
                        # TensorScalarPtr may read only one PSUM operand:
                        # bounce the broadcast recip through SBUF.
                        rbc = small.tile([D, 512], F32, tag="rbc", name="rbc")
                        nc.vector.tensor_copy(rbc[:], av[qb][64:128, :])
                        nc.vector.scalar_tensor_tensor(
                            out=aoT[dh][po:po + D, qb * 512:(qb + 1) * 512],
                            in0=av[qb][0:D, :],
                            scalar=1.0,
                            in1=rbc[:],
                            op0=MULT,
                            op1=MULT,
                        )

                def out_proj(qb, co):
                    ps = proj_ps.tile([P, 512], F32, tag="proj", name="ps_o")
                    for cb in range(CT):
                        nc.tensor.matmul(
                            ps[:],
                            wp[cb][:, co * P:(co + 1) * P],
                            aoT[cb][:, qb * 512:(qb + 1) * 512],
                            start=(cb == 0),
                            stop=(cb == CT - 1),
                        )
                    o = outst.tile([P, 512], F32, tag="outst", name="outst")
                    if with_bias:
                        nc.vector.tensor_scalar_add(o[:], ps[:], bp_col[co][:])
                    else:
                        nc.vector.tensor_copy(o[:], ps[:])
                    nc.sync.dma_start(
                        out=out[co * P:(co + 1) * P, qb * 512:(qb + 1) * 512],
                        in_=o[:],
                    )

                # ---- schedule ----
                bq_arg = bq_col if with_bias else None
                bk_arg = bk_col if with_bias else None

                def Qp(dh, qb):
                    proj_q(dh, qb, bq_arg)

                def Kp(dh, nb):
                    proj_k(dh, nb, bk_arg)

                # zero the sibling-head rows of the padded query tiles
                for h in range(H):
                    zo = (1 - h % 2) * D
                    nc.vector.memset(qTz[h][zo:zo + D, :], 0.0)

                # prologue: qT/kT for heads 0/1, logits+exp head 0
                for qb in range(QB):
                    Qp(0, qb)
                for nb in range(NB):
                    Kp(0, nb)
                    for kt in range(4 * nb, 4 * nb + 4):
                        logits_exp(0, kt)

                # Slot s is emitted while ACT chews exp(head s).  Each slot
                # feeds logits for head s+1 just-in-time, runs one lagged AV,
                # and drains a deadline-ordered queue of projection chunks
                # (qT/kT for later heads, V for AV).  Deadlines: kT[dh]
                # before logits(2dh) chunks; all vaug before av(0) in slot 2.
                extra = {
                    0: [lambda qb=qb: Qp(1, qb) for qb in range(QB)]
                    + [lambda nb=nb: Kp(1, nb) for nb in range(NB)],
                    1: [lambda nt=nt: proj_v(nt) for nt in range(0, 12)]
                    + [lambda qb=qb: Qp(2, qb) for qb in range(QB)],
                    2: [lambda nt=nt: proj_v(nt) for nt in range(12, 16)]
                    + [lambda nb=nb: Kp(2, nb) for nb in range(NB)],
                    3: [lambda qb=qb: Qp(3, qb) for qb in range(QB)]
                    + [lambda nb=nb: Kp(3, nb) for nb in range(NB)],
                    4: [], 5: [], 6: [], 7: [],
                }
                # av(h): (head, kt_lo, kt_hi) emission ranges per slot.
                av_sched = {
                    0: [], 1: [],
                    2: [(0, 0, KT)],
                    3: [(1, 0, KT)],
                    4: [(2, 0, KT)],
                    5: [(3, 0, KT)],
                    6: [(4, 0, KT), (5, 0, 8)],
                    7: [(5, 8, KT), (6, 0, KT), (7, 0, 8)],
                }
                av_live: dict[int, list] = {}

                def av_step(h, kt):
                    if h not in av_live:
                        av_live[h] = av_alloc()
                    av_mm(h, kt, av_live[h])
                    if kt == KT - 1:
                        av_fin(h, av_live.pop(h))

                for slot in range(H):
                    nxt = slot + 1
                    work = list(extra[slot])
                    avq = [(h, kt) for (h, lo, hi) in av_sched[slot]
                           for kt in range(lo, hi)]
                    # spread av steps evenly across the 8 logits pairs
                    per_i = (len(avq) + 7) // 8
                    for i in range(8):
                        if nxt < H:
                            logits_exp(nxt, 2 * i)
                            logits_exp(nxt, 2 * i + 1)
                        if work:
                            work.pop(0)()
                        for _ in range(per_i):
                            if avq:
                                av_step(*avq.pop(0))
                    while work:
                        work.pop(0)()
                    while avq:
                        av_step(*avq.pop(0))

                # tail: finish av(7), then output projection
                for kt in range(8, KT):
                    av_step(7, kt)
                for qb in range(QB):
                    for co in range(CT):
                        out_proj(qb, co)

    nc.compile()
    return nc


_CACHE: dict = {}


def get_nc(with_bias: bool):
    key = ("nc", with_bias)
    if key not in _CACHE:
        _CACHE[key] = build_nc(with_bias)
    return _CACHE[key]


def _bf16(a):
    import ml_dtypes
    return np.ascontiguousarray(a.astype(ml_dtypes.bfloat16))


def make_in_maps(inputs: dict) -> tuple[list[dict], bool]:
    x = np.asarray(inputs["x"], dtype=np.float32)
    mlp = np.asarray(inputs["mlp_out"], dtype=np.float32)
    Wq = np.asarray(inputs["Wq"], dtype=np.float32)
    Wk = np.asarray(inputs["Wk"], dtype=np.float32)
    Wv = np.asarray(inputs["Wv"], dtype=np.float32)
    Wp = np.asarray(inputs["Wp"], dtype=np.float32)
    bq = np.asarray(inputs["bq"], dtype=np.float32)
    bk = np.asarray(inputs["bk"], dtype=np.float32)
    bv = np.asarray(inputs["bv"], dtype=np.float32)
    bp = np.asarray(inputs["bp"], dtype=np.float32)

    with_bias = bool(np.any(bq) or np.any(bk) or np.any(bv) or np.any(bp))

    wqT = _bf16(Wq.T)
    wkT = _bf16(Wk.T)
    wvT = _bf16(Wv.T)
    wpT = _bf16(Wp.T)

    in_maps = []
    for c in range(NCORES):
        b, half = c // 2, c % 2
        xs = _bf16(x[b, half * QSH:(half + 1) * QSH, :].T)
        ms = _bf16(mlp[b].T)
        m = {
            "xT": xs, "mlpT": ms,
            "wqT": wqT, "wkT": wkT, "wvT": wvT, "wpT": wpT,
        }
        if with_bias:
            m["bqc"] = bq.reshape(C, 1)
            m["bkc"] = bk.reshape(C, 1)
            m["bvr"] = bv.reshape(1, C)
            m["bpc"] = bp.reshape(C, 1)
        in_maps.append(m)
    return in_maps, with_bias


def kernel(**inputs) -> np.ndarray:
    in_maps, with_bias = make_in_maps(inputs)
    nc = get_nc(with_bias)
    res = run_bass_kernel_spmd(nc, in_maps, list(range(NCORES)))
    full = np.empty((B, N, C), dtype=np.float32)
    for c in range(NCORES):
        b, half = c // 2, c % 2
        full[b, half * QSH:(half + 1) * QSH, :] = res.results[c]["out"].T
    return full


# revision 17
# speedup vs baseline: 1.5898x; 1.4592x over previous
"""Cross-attention kernel for Trainium2, SPMD across 8 NeuronCores.

Problem shapes (hardcoded): x [4, 2048, 512], mlp_out [4, 2048, 512],
Wq/Wk/Wv/Wp [512, 512], biases [512]. 8 heads x 64 head-dim.

Sharding: core c handles batch b = c//2 and query rows
[(c%2)*1024 : (c%2+1)*1024).  K/V work is duplicated across the two
cores of a batch pair; no collective is needed.

Design notes (HW-measured, not cost-model):
  - All SBUF data bf16 (halves DMA + SBUF); PSUM fp32.
  - K=64-contraction matmuls measure ~7x slower per instruction than
    K=128 on this HW, so the logits matmul pads the moving operand:
    per-head qTz tiles keep the sibling head's 64 partition rows at
    zero, letting the full two-head kT tile serve as a K=128
    stationary.
  - AV uses attnT as stationary ([k,q] tiles from exp) and the
    65-wide vaug (V plus a ones column) as moving, accumulating
    av[q, d+1] per (head, q-tile).  The ones column yields the
    softmax denominator per q ON PARTITIONS, so the division is one
    DVE reciprocal + tensor_scalar_mul (broadcast-free).  The
    alternative (out [d+1, q], PE-broadcast of the recip row)
    measured ~90us slower end-to-end from serialized chains.
  - Output projection: PE-transpose ao[q,c] chunks, then wp matmuls;
    output written [q, co] directly.
"""

import numpy as np

import concourse.bass as bass
import concourse.tile as tile
from concourse import bacc, mybir
from concourse.bass_utils import run_bass_kernel_spmd
from concourse.masks import make_identity

B = 4
N = 2048          # key/value sequence length
C = 512           # model dim
H = 8
D = C // H        # 64
NCORES = 8
QSH = N // 2      # query rows per core (1024)

F32 = mybir.dt.float32
F32R = mybir.dt.float32r
BF16 = mybir.dt.bfloat16

P = 128
CT = C // P       # 4 tiles along any model-dim axis
KT = N // P       # 16 key tiles
QT = QSH // P     # 8 query tiles
QB = QSH // 512   # 2 query 512-blocks
NB = N // 512     # 4 key 512-blocks
DHT = CT          # 4 dh tiles (2 heads each)


def build_nc(with_bias: bool, reps: int = 1):
    nc = bacc.Bacc("TRN2", target_bir_lowering=False, debug=False)

    xT = nc.dram_tensor("xT", [C, QSH], BF16, kind="ExternalInput")
    mlpT = nc.dram_tensor("mlpT", [C, N], BF16, kind="ExternalInput")
    wqT = nc.dram_tensor("wqT", [C, C], BF16, kind="ExternalInput")
    wkT = nc.dram_tensor("wkT", [C, C], BF16, kind="ExternalInput")
    wvT = nc.dram_tensor("wvT", [C, C], BF16, kind="ExternalInput")
    wpT = nc.dram_tensor("wpT", [C, C], BF16, kind="ExternalInput")
    if with_bias:
        bqc = nc.dram_tensor("bqc", [C, 1], F32, kind="ExternalInput")
        bkc = nc.dram_tensor("bkc", [C, 1], F32, kind="ExternalInput")
        bvr = nc.dram_tensor("bvr", [1, C], F32, kind="ExternalInput")
        bpr = nc.dram_tensor("bpr", [1, C], F32, kind="ExternalInput")
    out = nc.dram_tensor("out", [QSH, C], F32, kind="ExternalOutput")

    with tile.TileContext(nc) as tc:
        from contextlib import ExitStack

        with ExitStack() as ctx:
            const = ctx.enter_context(tc.tile_pool(name="const", bufs=1))
            w_pool = ctx.enter_context(tc.tile_pool(name="w", bufs=1))
            x_pool = ctx.enter_context(tc.tile_pool(name="x", bufs=1))
            m_pool = ctx.enter_context(tc.tile_pool(name="mlp", bufs=1))
            qt_pool = ctx.enter_context(tc.tile_pool(name="qT", bufs=1))
            kt_pool = ctx.enter_context(tc.tile_pool(name="kT", bufs=1))
            v_pool = ctx.enter_context(tc.tile_pool(name="vaug", bufs=1))
            attn_pool = ctx.enter_context(tc.tile_pool(name="attnT", bufs=40))
            ao_pool = ctx.enter_context(tc.tile_pool(name="aoQ", bufs=1))
            aoTq_pool = ctx.enter_context(tc.tile_pool(name="aoTq", bufs=2))
            outst = ctx.enter_context(tc.tile_pool(name="outst", bufs=2))
            small = ctx.enter_context(tc.tile_pool(name="small", bufs=4))
            lp_ps = ctx.enter_context(
                tc.tile_pool(name="lp_ps", bufs=2, space="PSUM")
            )
            av_ps = ctx.enter_context(
                tc.tile_pool(name="av_ps", bufs=2, space="PSUM")
            )
            proj_ps = ctx.enter_context(
                tc.tile_pool(name="proj_ps", bufs=2, space="PSUM")
            )

            ident = const.tile([P, P], F32)
            make_identity(nc, ident)
            ident_bf = const.tile([P, P], BF16)
            nc.vector.tensor_copy(ident_bf[:], ident[:])
            ones_f = const.tile([1, P], F32)
            nc.vector.memset(ones_f[:], 1.0)
            ones_r = const.tile([1, P], F32R)
            nc.vector.tensor_copy(ones_r[:], ones_f[:])

            for _rep in range(reps):

                # ---- resident tiles ----
                wq = [w_pool.tile([P, C], BF16, tag=f"wq{i}", name=f"wq{i}")
                      for i in range(CT)]
                wk = [w_pool.tile([P, C], BF16, tag=f"wk{i}", name=f"wk{i}")
                      for i in range(CT)]
                wv = [w_pool.tile([P, C], BF16, tag=f"wv{i}", name=f"wv{i}")
                      for i in range(CT)]
                wp = [w_pool.tile([P, C], BF16, tag=f"wp{i}", name=f"wp{i}")
                      for i in range(CT)]
                xt = [x_pool.tile([P, QSH], BF16, tag=f"x{i}", name=f"x{i}")
                      for i in range(CT)]
                mt = [m_pool.tile([P, N], BF16, tag=f"m{i}", name=f"m{i}")
                      for i in range(CT)]
                # per-head query tiles, sibling head's rows zeroed (K=128
                # logits trick)
                qTz = [qt_pool.tile([P, QSH], BF16, tag=f"qz{h}", name=f"qz{h}")
                       for h in range(H)]
                kT = [kt_pool.tile([P, N], BF16, tag=f"kT{i}", name=f"kT{i}")
                      for i in range(DHT)]
                vaug = [v_pool.tile([P, H, D + 1], BF16, tag=f"v{i}",
                                    name=f"v{i}") for i in range(KT)]
                aoQ = [ao_pool.tile([P, C], BF16, tag=f"ao{i}", name=f"ao{i}")
                       for i in range(QT)]

                # ---- DMA prologue, critical-path first ----
                def dma_w(tiles, dram):
                    for i, t in enumerate(tiles):
                        nc.sync.dma_start(out=t[:], in_=dram[i * P:(i + 1) * P, :])

                def dma_chunk(t, dram, row0, col0, cols):
                    nc.sync.dma_start(
                        out=t[:, col0:col0 + cols],
                        in_=dram[row0:row0 + P, col0:col0 + cols],
                    )

                dma_w(wq, wqT)
                for i in range(CT):          # x qb0 chunks
                    dma_chunk(xt[i], xT, i * P, 0, 512)
                dma_w(wk, wkT)
                for i in range(CT):          # mlp nb0 chunks
                    dma_chunk(mt[i], mlpT, i * P, 0, 512)
                for i in range(CT):          # x qb1
                    dma_chunk(xt[i], xT, i * P, 512, 512)
                for nb in range(1, NB):      # rest of mlp
                    for i in range(CT):
                        dma_chunk(mt[i], mlpT, i * P, nb * 512, 512)
                dma_w(wv, wvT)
                dma_w(wp, wpT)

                if with_bias:
                    bq_col = [small.tile([P, 1], F32, tag=f"bq{i}", name=f"bq{i}")
                              for i in range(DHT)]
                    bk_col = [small.tile([P, 1], F32, tag=f"bk{i}", name=f"bk{i}")
                              for i in range(DHT)]
                    for i in range(DHT):
                        nc.sync.dma_start(out=bq_col[i][:],
                                          in_=bqc[i * P:(i + 1) * P, :])
                        nc.sync.dma_start(out=bk_col[i][:],
                                          in_=bkc[i * P:(i + 1) * P, :])
                    def load_row_r(dram_row, nm):
                        f = const.tile([1, C], F32, name=f"{nm}_f")
                        nc.sync.dma_start(out=f[:], in_=dram_row[:])
                        r = const.tile([1, C], F32R, name=f"{nm}_r")
                        nc.vector.tensor_copy(r[:], f[:])
                        return r
                    bv_r = load_row_r(bvr, "bv")
                    bp_r = load_row_r(bpr, "bp")

                # ---- work chunks ----
                def _proj_mm(wt, dh, src, nb_):
                    ps = proj_ps.tile([P, 512], F32, tag="proj", name="ps_qk")
                    for cb in range(CT):
                        nc.tensor.matmul(
                            ps[:],
                            wt[cb][:, dh * P:(dh + 1) * P],
                            src[cb][:, nb_ * 512:(nb_ + 1) * 512],
                            start=(cb == 0),
                            stop=(cb == CT - 1),
                        )
                    return ps

                def proj_q(dh, qb):
                    # evict each head's 64 rows into its zero-padded tile
                    ps = _proj_mm(wq, dh, xt, qb)
                    for par in range(2):
                        h = 2 * dh + par
                        po = par * D
                        sl = qTz[h][po:po + D, qb * 512:(qb + 1) * 512]
                        if with_bias:
                            nc.vector.tensor_scalar_add(
                                sl, ps[po:po + D, :],
                                bq_col[dh][po:po + D, :])
                        else:
                            nc.vector.tensor_copy(sl, ps[po:po + D, :])

                def proj_k(dh, nb_):
                    ps = _proj_mm(wk, dh, mt, nb_)
                    sl = kT[dh][:, nb_ * 512:(nb_ + 1) * 512]
                    if with_bias:
                        nc.vector.tensor_scalar_add(sl, ps[:], bk_col[dh][:])
                    else:
                        nc.vector.tensor_copy(sl, ps[:])

                def proj_v(nt):
                    nb_ = nt // 4
                    ps = proj_ps.tile([P, 512], F32, tag="proj", name="ps_v")
                    for cb in range(CT):
                        nc.tensor.matmul(
                            ps[:],
                            mt[cb][:, nt * P:(nt + 1) * P],
                            wv[cb][:],
                            start=(cb == 0),
                            stop=(cb == CT - 1 and not with_bias),
                        )
                    if with_bias:
                        nc.tensor.matmul(ps[:], ones_r[:], bv_r[:],
                                         start=False, stop=True)
                    nc.vector.tensor_copy(
                        vaug[nt][:, :, 0:D],
                        ps[:].rearrange("p (h d) -> p h d", h=H),
                    )
                    nc.vector.memset(vaug[nt][:, :, D:D + 1], 1.0)

                attnT = {h: [] for h in range(H)}

                def logits_exp(h, kt):
                    dh = h // 2
                    lp = lp_ps.tile([P, QSH], F32, tag="lp", name="lp")
                    for qb in range(QB):
                        nc.tensor.matmul(
                            lp[:, qb * 512:(qb + 1) * 512],
                            kT[dh][:, kt * P:(kt + 1) * P],
                            qTz[h][:, qb * 512:(qb + 1) * 512],
                            start=True,
                            stop=True,
                        )
                    at = attn_pool.tile([P, QSH], BF16, tag="attnT", name="at")
                    nc.scalar.activation(
                        out=at[:], in_=lp[:],
                        func=mybir.ActivationFunctionType.Exp,
                    )
                    attnT[h].append(at)

                def av_group(h, qt):
                    # av[q, d+1] for one (head, q-tile); the ones column of
                    # vaug gives the softmax denominator on partitions.
                    av = av_ps.tile([P, D + 1], F32, tag="av", name="av")
                    tiles = attnT[h]
                    for kt in range(KT):
                        nc.tensor.matmul(
                            av[:],
                            tiles[kt][:, qt * P:(qt + 1) * P],
                            vaug[kt][:, h, :],
                            start=(kt == 0),
                            stop=(kt == KT - 1),
                        )
                    recip = small.tile([P, 1], F32, tag="recip", name="recip")
                    nc.vector.reciprocal(recip[:], av[:, D:D + 1])
                    nc.vector.tensor_scalar_mul(
                        aoQ[qt][:, h * D:(h + 1) * D], av[:, 0:D], recip[:])

                def tail_qt(qt):
                    # transpose ao[q, c] chunks, then project: out[q, co]
                    ps_t = proj_ps.tile([P, 512], BF16, tag="proj", name="ps_t")
                    for mt_ in range(CT):
                        nc.tensor.transpose(
                            ps_t[:, mt_ * P:(mt_ + 1) * P],
                            aoQ[qt][:, mt_ * P:(mt_ + 1) * P],
                            ident_bf[:],
                        )
                    aoTq = aoTq_pool.tile([P, CT, P], BF16, tag="aoTq",
                                          name="aoTq")
                    nc.vector.tensor_copy(
                        aoTq[:], ps_t[:].rearrange("p (m q) -> p m q", m=CT))
                    po = proj_ps.tile([P, 512], F32, tag="proj", name="ps_o")
                    for mt_ in range(CT):
                        nc.tensor.matmul(
                            po[:],
                            aoTq[:, mt_, :],
                            wp[mt_][:],
                            start=(mt_ == 0),
                            stop=(mt_ == CT - 1 and not with_bias),
                        )
                    if with_bias:
                        nc.tensor.matmul(po[:], ones_r[:], bp_r[:],
                                         start=False, stop=True)
                    o = outst.tile([P, C], F32, tag="outst", name="outst")
                    nc.vector.tensor_copy(o[:], po[:])
                    nc.sync.dma_start(out=out[qt * P:(qt + 1) * P, :], in_=o[:])

                # ---- schedule ----
                def Qp(dh, qb):
                    proj_q(dh, qb)

                def Kp(dh, nb):
                    proj_k(dh, nb)

                # zero the sibling-head rows of the padded query tiles
                for h in range(H):
                    zo = (1 - h % 2) * D
                    nc.vector.memset(qTz[h][zo:zo + D, :], 0.0)

                # prologue: qT/kT for heads 0/1, logits+exp head 0
                for qb in range(QB):
                    Qp(0, qb)
                for nb in range(NB):
                    Kp(0, nb)
                    for kt in range(4 * nb, 4 * nb + 4):
                        logits_exp(0, kt)

                # Slot s feeds logits for head s+1, runs AV groups of lagged
                # heads, and drains deadline-ordered projection chunks.
                # All proj_v chunks must be emitted before the first av_group
                # (slot 2): each av_group reads every vaug tile.
                extra = {
                    0: [lambda qb=qb: Qp(1, qb) for qb in range(QB)]
                    + [lambda nb=nb: Kp(1, nb) for nb in range(NB)]
                    + [lambda nt=nt: proj_v(nt) for nt in range(0, 4)],
                    1: [lambda nt=nt: proj_v(nt) for nt in range(4, 16)]
                    + [lambda qb=qb: Qp(2, qb) for qb in range(QB)]
                    + [lambda nb=nb: Kp(2, nb) for nb in range(NB)],
                    2: [lambda qb=qb: Qp(3, qb) for qb in range(QB)]
                    + [lambda nb=nb: Kp(3, nb) for nb in range(NB)],
                    3: [], 4: [], 5: [], 6: [], 7: [],
                }
                # av groups (head, q-tile) per slot; av(h) runs in slot h+2
                av_sched = {
                    0: [], 1: [],
                    2: [(0, qt) for qt in range(QT)],
                    3: [(1, qt) for qt in range(QT)],
                    4: [(2, qt) for qt in range(QT)],
                    5: [(3, qt) for qt in range(QT)],
                    6: [(4, qt) for qt in range(QT)]
                    + [(5, qt) for qt in range(QT // 2)],
                    7: [(5, qt) for qt in range(QT // 2, QT)]
                    + [(6, qt) for qt in range(QT)]
                    + [(7, qt) for qt in range(QT // 2)],
                }

                for slot in range(H):
                    nxt = slot + 1
                    work = list(extra[slot])
                    avq = list(av_sched[slot])
                    per_i = (len(avq) + 7) // 8
                    for i in range(8):
                        if nxt < H:
                            logits_exp(nxt, 2 * i)
                            logits_exp(nxt, 2 * i + 1)
                        if work:
                            work.pop(0)()
                        for _ in range(per_i):
                            if avq:
                                h_, qt_ = avq.pop(0)
                                av_group(h_, qt_)
                    while work:
                        work.pop(0)()
                    for h_, qt_ in avq:
                        av_group(h_, qt_)

                # tail: finish av(7), then per-qtile output projection
                for qt in range(QT // 2, QT):
                    av_group(7, qt)
                for qt in range(QT):
                    tail_qt(qt)

    nc.compile()
    return nc


_CACHE: dict = {}


def get_nc(with_bias: bool):
    key = ("nc", with_bias)
    if key not in _CACHE:
        _CACHE[key] = build_nc(with_bias)
    return _CACHE[key]


def _bf16(a):
    import ml_dtypes
    return np.ascontiguousarray(a.astype(ml_dtypes.bfloat16))


def make_in_maps(inputs: dict) -> tuple[list[dict], bool]:
    x = np.asarray(inputs["x"], dtype=np.float32)
    mlp = np.asarray(inputs["mlp_out"], dtype=np.float32)
    Wq = np.asarray(inputs["Wq"], dtype=np.float32)
    Wk = np.asarray(inputs["Wk"], dtype=np.float32)
    Wv = np.asarray(inputs["Wv"], dtype=np.float32)
    Wp = np.asarray(inputs["Wp"], dtype=np.float32)
    bq = np.asarray(inputs["bq"], dtype=np.float32)
    bk = np.asarray(inputs["bk"], dtype=np.float32)
    bv = np.asarray(inputs["bv"], dtype=np.float32)
    bp = np.asarray(inputs["bp"], dtype=np.float32)

    with_bias = bool(np.any(bq) or np.any(bk) or np.any(bv) or np.any(bp))

    wqT = _bf16(Wq.T)
    wkT = _bf16(Wk.T)
    wvT = _bf16(Wv.T)
    wpT = _bf16(Wp.T)

    in_maps = []
    for c in range(NCORES):
        b, half = c // 2, c % 2
        xs = _bf16(x[b, half * QSH:(half + 1) * QSH, :].T)
        ms = _bf16(mlp[b].T)
        m = {
            "xT": xs, "mlpT": ms,
            "wqT": wqT, "wkT": wkT, "wvT": wvT, "wpT": wpT,
        }
        if with_bias:
            m["bqc"] = bq.reshape(C, 1)
            m["bkc"] = bk.reshape(C, 1)
            m["bvr"] = bv.reshape(1, C)
            m["bpr"] = bp.reshape(1, C)
        in_maps.append(m)
    return in_maps, with_bias


def kernel(**inputs) -> np.ndarray:
    in_maps, with_bias = make_in_maps(inputs)
    nc = get_nc(with_bias)
    res = run_bass_kernel_spmd(nc, in_maps, list(range(NCORES)))
    full = np.empty((B, N, C), dtype=np.float32)
    for c in range(NCORES):
        b, half = c // 2, c % 2
        full[b, half * QSH:(half + 1) * QSH, :] = res.results[c]["out"]
    return full


# revision 19
# speedup vs baseline: 1.6800x; 1.0567x over previous
"""Cross-attention kernel for Trainium2, SPMD across 8 NeuronCores.

Problem shapes (hardcoded): x [4, 2048, 512], mlp_out [4, 2048, 512],
Wq/Wk/Wv/Wp [512, 512], biases [512]. 8 heads x 64 head-dim.

Sharding: core c handles batch b = c//2 and query rows
[(c%2)*1024 : (c%2+1)*1024).  K/V work is duplicated across the two
cores of a batch pair; no collective is needed.

Design notes (HW-measured, not cost-model):
  - All SBUF data bf16 (halves DMA + SBUF); PSUM fp32.
  - K=64-contraction matmuls measure ~7x slower per instruction than
    K=128 on this HW, so the logits matmul pads the moving operand:
    per-head qTz tiles keep the sibling head's 64 partition rows at
    zero, letting the full two-head kT tile serve as a K=128
    stationary.
  - AV uses attnT as stationary ([k,q] tiles from exp) and the
    65-wide vaug (V plus a ones column) as moving, accumulating
    av[q, d+1] per (head, q-tile).  The ones column yields the
    softmax denominator per q ON PARTITIONS, so the division is one
    DVE reciprocal + tensor_scalar_mul (broadcast-free).  The
    alternative (out [d+1, q], PE-broadcast of the recip row)
    measured ~90us slower end-to-end from serialized chains.
  - Output projection: PE-transpose ao[q,c] chunks, then wp matmuls;
    output written [q, co] directly.
"""

import numpy as np

import concourse.bass as bass
import concourse.tile as tile
from concourse import bacc, mybir
from concourse.bass_utils import run_bass_kernel_spmd
from concourse.masks import make_identity

B = 4
N = 2048          # key/value sequence length
C = 512           # model dim
H = 8
D = C // H        # 64
NCORES = 8
QSH = N // 2      # query rows per core (1024)

F32 = mybir.dt.float32
F32R = mybir.dt.float32r
BF16 = mybir.dt.bfloat16

P = 128
CT = C // P       # 4 tiles along any model-dim axis
KT = N // P       # 16 key tiles
QT = QSH // P     # 8 query tiles
QB = QSH // 512   # 2 query 512-blocks
NB = N // 512     # 4 key 512-blocks
DHT = CT          # 4 dh tiles (2 heads each)


def build_nc(with_bias: bool, reps: int = 1):
    nc = bacc.Bacc("TRN2", target_bir_lowering=False, debug=False)

    xT = nc.dram_tensor("xT", [C, QSH], BF16, kind="ExternalInput")
    mlpT = nc.dram_tensor("mlpT", [C, N], BF16, kind="ExternalInput")
    wqT = nc.dram_tensor("wqT", [C, C], BF16, kind="ExternalInput")
    wkT = nc.dram_tensor("wkT", [C, C], BF16, kind="ExternalInput")
    wvT = nc.dram_tensor("wvT", [C, C], BF16, kind="ExternalInput")
    wpT = nc.dram_tensor("wpT", [C, C], BF16, kind="ExternalInput")
    if with_bias:
        bqc = nc.dram_tensor("bqc", [C, 1], F32, kind="ExternalInput")
        bkc = nc.dram_tensor("bkc", [C, 1], F32, kind="ExternalInput")
        bvr = nc.dram_tensor("bvr", [1, C], F32, kind="ExternalInput")
        bpr = nc.dram_tensor("bpr", [1, C], F32, kind="ExternalInput")
    out = nc.dram_tensor("out", [QSH, C], F32, kind="ExternalOutput")

    with tile.TileContext(nc) as tc:
        from contextlib import ExitStack

        with ExitStack() as ctx:
            const = ctx.enter_context(tc.tile_pool(name="const", bufs=1))
            w_pool = ctx.enter_context(tc.tile_pool(name="w", bufs=1))
            x_pool = ctx.enter_context(tc.tile_pool(name="x", bufs=1))
            m_pool = ctx.enter_context(tc.tile_pool(name="mlp", bufs=1))
            qt_pool = ctx.enter_context(tc.tile_pool(name="qT", bufs=1))
            kt_pool = ctx.enter_context(tc.tile_pool(name="kT", bufs=1))
            v_pool = ctx.enter_context(tc.tile_pool(name="vaug", bufs=1))
            attn_pool = ctx.enter_context(tc.tile_pool(name="attnT", bufs=40))
            ao_pool = ctx.enter_context(tc.tile_pool(name="aoQ", bufs=1))
            aoTq_pool = ctx.enter_context(tc.tile_pool(name="aoTq", bufs=2))
            outst = ctx.enter_context(tc.tile_pool(name="outst", bufs=2))
            small = ctx.enter_context(tc.tile_pool(name="small", bufs=4))
            lp_ps = ctx.enter_context(
                tc.tile_pool(name="lp_ps", bufs=4, space="PSUM")
            )
            av_ps = ctx.enter_context(
                tc.tile_pool(name="av_ps", bufs=2, space="PSUM")
            )
            proj_ps = ctx.enter_context(
                tc.tile_pool(name="proj_ps", bufs=2, space="PSUM")
            )

            ident = const.tile([P, P], F32)
            make_identity(nc, ident)
            ident_bf = const.tile([P, P], BF16)
            nc.vector.tensor_copy(ident_bf[:], ident[:])
            ones_f = const.tile([1, P], F32)
            nc.vector.memset(ones_f[:], 1.0)
            ones_r = const.tile([1, P], F32R)
            nc.vector.tensor_copy(ones_r[:], ones_f[:])

            for _rep in range(reps):

                # ---- resident tiles ----
                wq = [w_pool.tile([P, C], BF16, tag=f"wq{i}", name=f"wq{i}")
                      for i in range(CT)]
                wk = [w_pool.tile([P, C], BF16, tag=f"wk{i}", name=f"wk{i}")
                      for i in range(CT)]
                wv = [w_pool.tile([P, C], BF16, tag=f"wv{i}", name=f"wv{i}")
                      for i in range(CT)]
                wp = [w_pool.tile([P, C], BF16, tag=f"wp{i}", name=f"wp{i}")
                      for i in range(CT)]
                xt = [x_pool.tile([P, QSH], BF16, tag=f"x{i}", name=f"x{i}")
                      for i in range(CT)]
                mt = [m_pool.tile([P, N], BF16, tag=f"m{i}", name=f"m{i}")
                      for i in range(CT)]
                # per-head query tiles, sibling head's rows zeroed (K=128
                # logits trick)
                qTz = [qt_pool.tile([P, QSH], BF16, tag=f"qz{h}", name=f"qz{h}")
                       for h in range(H)]
                kT = [kt_pool.tile([P, N], BF16, tag=f"kT{i}", name=f"kT{i}")
                      for i in range(DHT)]
                vaug = [v_pool.tile([P, H, D + 1], BF16, tag=f"v{i}",
                                    name=f"v{i}") for i in range(KT)]
                aoQ = [ao_pool.tile([P, C], BF16, tag=f"ao{i}", name=f"ao{i}")
                       for i in range(QT)]

                # ---- DMA prologue, critical-path first ----
                def dma_w(tiles, dram):
                    for i, t in enumerate(tiles):
                        nc.sync.dma_start(out=t[:], in_=dram[i * P:(i + 1) * P, :])

                def dma_chunk(t, dram, row0, col0, cols):
                    nc.sync.dma_start(
                        out=t[:, col0:col0 + cols],
                        in_=dram[row0:row0 + P, col0:col0 + cols],
                    )

                dma_w(wq, wqT)
                for i in range(CT):          # x qb0 chunks
                    dma_chunk(xt[i], xT, i * P, 0, 512)
                dma_w(wk, wkT)
                for i in range(CT):          # mlp nb0 chunks
                    dma_chunk(mt[i], mlpT, i * P, 0, 512)
                for i in range(CT):          # x qb1
                    dma_chunk(xt[i], xT, i * P, 512, 512)
                for nb in range(1, NB):      # rest of mlp
                    for i in range(CT):
                        dma_chunk(mt[i], mlpT, i * P, nb * 512, 512)
                dma_w(wv, wvT)
                dma_w(wp, wpT)

                if with_bias:
                    bq_col = [small.tile([P, 1], F32, tag=f"bq{i}", name=f"bq{i}")
                              for i in range(DHT)]
                    bk_col = [small.tile([P, 1], F32, tag=f"bk{i}", name=f"bk{i}")
                              for i in range(DHT)]
                    for i in range(DHT):
                        nc.sync.dma_start(out=bq_col[i][:],
                                          in_=bqc[i * P:(i + 1) * P, :])
                        nc.sync.dma_start(out=bk_col[i][:],
                                          in_=bkc[i * P:(i + 1) * P, :])
                    def load_row_r(dram_row, nm):
                        f = const.tile([1, C], F32, name=f"{nm}_f")
                        nc.sync.dma_start(out=f[:], in_=dram_row[:])
                        r = const.tile([1, C], F32R, name=f"{nm}_r")
                        nc.vector.tensor_copy(r[:], f[:])
                        return r
                    bv_r = load_row_r(bvr, "bv")
                    bp_r = load_row_r(bpr, "bp")

                # ---- work chunks ----
                def _proj_mm(wt, dh, src, nb_):
                    ps = proj_ps.tile([P, 512], F32, tag="proj", name="ps_qk")
                    for cb in range(CT):
                        nc.tensor.matmul(
                            ps[:],
                            wt[cb][:, dh * P:(dh + 1) * P],
                            src[cb][:, nb_ * 512:(nb_ + 1) * 512],
                            start=(cb == 0),
                            stop=(cb == CT - 1),
                        )
                    return ps

                def proj_q(dh, qb):
                    # evict each head's 64 rows into its zero-padded tile
                    ps = _proj_mm(wq, dh, xt, qb)
                    for par in range(2):
                        h = 2 * dh + par
                        po = par * D
                        sl = qTz[h][po:po + D, qb * 512:(qb + 1) * 512]
                        if with_bias:
                            nc.vector.tensor_scalar_add(
                                sl, ps[po:po + D, :],
                                bq_col[dh][po:po + D, :])
                        else:
                            nc.vector.tensor_copy(sl, ps[po:po + D, :])

                def proj_k(dh, nb_):
                    ps = _proj_mm(wk, dh, mt, nb_)
                    sl = kT[dh][:, nb_ * 512:(nb_ + 1) * 512]
                    if with_bias:
                        nc.vector.tensor_scalar_add(sl, ps[:], bk_col[dh][:])
                    else:
                        nc.vector.tensor_copy(sl, ps[:])

                def proj_v(nt):
                    nb_ = nt // 4
                    ps = proj_ps.tile([P, 512], F32, tag="proj", name="ps_v")
                    for cb in range(CT):
                        nc.tensor.matmul(
                            ps[:],
                            mt[cb][:, nt * P:(nt + 1) * P],
                            wv[cb][:],
                            start=(cb == 0),
                            stop=(cb == CT - 1 and not with_bias),
                        )
                    if with_bias:
                        nc.tensor.matmul(ps[:], ones_r[:], bv_r[:],
                                         start=False, stop=True)
                    nc.vector.tensor_copy(
                        vaug[nt][:, :, 0:D],
                        ps[:].rearrange("p (h d) -> p h d", h=H),
                    )
                    nc.vector.memset(vaug[nt][:, :, D:D + 1], 1.0)

                attnT = {h: [] for h in range(H)}

                def logits_exp(h, kt):
                    dh = h // 2
                    at = attn_pool.tile([P, QSH], BF16, tag="attnT", name="at")
                    for qb in range(QB):
                        lp = lp_ps.tile([P, 512], F32, tag="lp", name="lp")
                        nc.tensor.matmul(
                            lp[:],
                            kT[dh][:, kt * P:(kt + 1) * P],
                            qTz[h][:, qb * 512:(qb + 1) * 512],
                            start=True,
                            stop=True,
                        )
                        nc.scalar.activation(
                            out=at[:, qb * 512:(qb + 1) * 512], in_=lp[:],
                            func=mybir.ActivationFunctionType.Exp,
                        )
                    attnT[h].append(at)

                def av_group(h, qt):
                    # av[q, d+1] for one (head, q-tile); the ones column of
                    # vaug gives the softmax denominator on partitions.
                    av = av_ps.tile([P, D + 1], F32, tag="av", name="av")
                    tiles = attnT[h]
                    for kt in range(KT):
                        nc.tensor.matmul(
                            av[:],
                            tiles[kt][:, qt * P:(qt + 1) * P],
                            vaug[kt][:, h, :],
                            start=(kt == 0),
                            stop=(kt == KT - 1),
                        )
                    recip = small.tile([P, 1], F32, tag="recip", name="recip")
                    nc.vector.reciprocal(recip[:], av[:, D:D + 1])
                    nc.vector.tensor_scalar_mul(
                        aoQ[qt][:, h * D:(h + 1) * D], av[:, 0:D], recip[:])

                def tail_qt(qt):
                    # transpose ao[q, c] chunks, then project: out[q, co]
                    ps_t = proj_ps.tile([P, 512], BF16, tag="proj", name="ps_t")
                    for mt_ in range(CT):
                        nc.tensor.transpose(
                            ps_t[:, mt_ * P:(mt_ + 1) * P],
                            aoQ[qt][:, mt_ * P:(mt_ + 1) * P],
                            ident_bf[:],
                        )
                    aoTq = aoTq_pool.tile([P, CT, P], BF16, tag="aoTq",
                                          name="aoTq")
                    nc.vector.tensor_copy(
                        aoTq[:], ps_t[:].rearrange("p (m q) -> p m q", m=CT))
                    po = proj_ps.tile([P, 512], F32, tag="proj", name="ps_o")
                    for mt_ in range(CT):
                        nc.tensor.matmul(
                            po[:],
                            aoTq[:, mt_, :],
                            wp[mt_][:],
                            start=(mt_ == 0),
                            stop=(mt_ == CT - 1 and not with_bias),
                        )
                    if with_bias:
                        nc.tensor.matmul(po[:], ones_r[:], bp_r[:],
                                         start=False, stop=True)
                    o = outst.tile([P, C], F32, tag="outst", name="outst")
                    nc.vector.tensor_copy(o[:], po[:])
                    nc.sync.dma_start(out=out[qt * P:(qt + 1) * P, :], in_=o[:])

                # ---- schedule ----
                def Qp(dh, qb):
                    proj_q(dh, qb)

                def Kp(dh, nb):
                    proj_k(dh, nb)

                # zero the sibling-head rows of the padded query tiles
                for h in range(H):
                    zo = (1 - h % 2) * D
                    nc.vector.memset(qTz[h][zo:zo + D, :], 0.0)

                # prologue: qT/kT for heads 0/1, logits+exp head 0
                for qb in range(QB):
                    Qp(0, qb)
                for nb in range(NB):
                    Kp(0, nb)
                    for kt in range(4 * nb, 4 * nb + 4):
                        logits_exp(0, kt)

                # Slot s feeds logits for head s+1, runs AV groups of lagged
                # heads, and drains deadline-ordered projection chunks.
                # All proj_v chunks must be emitted before the first av_group
                # (slot 2): each av_group reads every vaug tile.
                extra = {
                    0: [lambda qb=qb: Qp(1, qb) for qb in range(QB)]
                    + [lambda nb=nb: Kp(1, nb) for nb in range(NB)]
                    + [lambda nt=nt: proj_v(nt) for nt in range(0, 4)],
                    1: [lambda nt=nt: proj_v(nt) for nt in range(4, 16)]
                    + [lambda qb=qb: Qp(2, qb) for qb in range(QB)]
                    + [lambda nb=nb: Kp(2, nb) for nb in range(NB)],
                    2: [lambda qb=qb: Qp(3, qb) for qb in range(QB)]
                    + [lambda nb=nb: Kp(3, nb) for nb in range(NB)],
                    3: [], 4: [], 5: [], 6: [], 7: [],
                }
                # av groups (head, q-tile) per slot; av(h) runs in slot h+2
                av_sched = {
                    0: [], 1: [],
                    2: [(0, qt) for qt in range(QT)],
                    3: [(1, qt) for qt in range(QT)],
                    4: [(2, qt) for qt in range(QT)],
                    5: [(3, qt) for qt in range(QT)],
                    6: [(4, qt) for qt in range(QT)]
                    + [(5, qt) for qt in range(QT // 2)],
                    7: [(5, qt) for qt in range(QT // 2, QT)]
                    + [(6, qt) for qt in range(QT)]
                    + [(7, qt) for qt in range(QT // 2)],
                }

                for slot in range(H):
                    nxt = slot + 1
                    work = list(extra[slot])
                    avq = list(av_sched[slot])
                    per_i = (len(avq) + 7) // 8
                    for i in range(8):
                        if nxt < H:
                            logits_exp(nxt, 2 * i)
                            logits_exp(nxt, 2 * i + 1)
                        if work:
                            work.pop(0)()
                        for _ in range(per_i):
                            if avq:
                                h_, qt_ = avq.pop(0)
                                av_group(h_, qt_)
                    while work:
                        work.pop(0)()
                    for h_, qt_ in avq:
                        av_group(h_, qt_)

                # tail: finish av(7), then per-qtile output projection
                for qt in range(QT // 2, QT):
                    av_group(7, qt)
                for qt in range(QT):
                    tail_qt(qt)

    nc.compile()
    return nc


_CACHE: dict = {}


def get_nc(with_bias: bool):
    key = ("nc", with_bias)
    if key not in _CACHE:
        _CACHE[key] = build_nc(with_bias)
    return _CACHE[key]


def _bf16(a):
    import ml_dtypes
    return np.ascontiguousarray(a.astype(ml_dtypes.bfloat16))


def make_in_maps(inputs: dict) -> tuple[list[dict], bool]:
    x = np.asarray(inputs["x"], dtype=np.float32)
    mlp = np.asarray(inputs["mlp_out"], dtype=np.float32)
    Wq = np.asarray(inputs["Wq"], dtype=np.float32)
    Wk = np.asarray(inputs["Wk"], dtype=np.float32)
    Wv = np.asarray(inputs["Wv"], dtype=np.float32)
    Wp = np.asarray(inputs["Wp"], dtype=np.float32)
    bq = np.asarray(inputs["bq"], dtype=np.float32)
    bk = np.asarray(inputs["bk"], dtype=np.float32)
    bv = np.asarray(inputs["bv"], dtype=np.float32)
    bp = np.asarray(inputs["bp"], dtype=np.float32)

    with_bias = bool(np.any(bq) or np.any(bk) or np.any(bv) or np.any(bp))

    wqT = _bf16(Wq.T)
    wkT = _bf16(Wk.T)
    wvT = _bf16(Wv.T)
    wpT = _bf16(Wp.T)

    in_maps = []
    for c in range(NCORES):
        b, half = c // 2, c % 2
        xs = _bf16(x[b, half * QSH:(half + 1) * QSH, :].T)
        ms = _bf16(mlp[b].T)
        m = {
            "xT": xs, "mlpT": ms,
            "wqT": wqT, "wkT": wkT, "wvT": wvT, "wpT": wpT,
        }
        if with_bias:
            m["bqc"] = bq.reshape(C, 1)
            m["bkc"] = bk.reshape(C, 1)
            m["bvr"] = bv.reshape(1, C)
            m["bpr"] = bp.reshape(1, C)
        in_maps.append(m)
    return in_maps, with_bias


def kernel(**inputs) -> np.ndarray:
    in_maps, with_bias = make_in_maps(inputs)
    nc = get_nc(with_bias)
    res = run_bass_kernel_spmd(nc, in_maps, list(range(NCORES)))
    full = np.empty((B, N, C), dtype=np.float32)
    for c in range(NCORES):
        b, half = c // 2, c % 2
        full[b, half * QSH:(half + 1) * QSH, :] = res.results[c]["out"]
    return full


# revision 21
# speedup vs baseline: 1.7584x; 1.0466x over previous
"""Cross-attention kernel for Trainium2, SPMD across 8 NeuronCores.

Problem shapes (hardcoded): x [4, 2048, 512], mlp_out [4, 2048, 512],
Wq/Wk/Wv/Wp [512, 512], biases [512]. 8 heads x 64 head-dim.

Sharding: core c handles batch b = c//2 and query rows
[(c%2)*1024 : (c%2+1)*1024).  K/V work is duplicated across the two
cores of a batch pair; no collective is needed.

Design notes (HW-measured, not cost-model):
  - All SBUF data bf16 (halves DMA + SBUF); PSUM fp32.
  - K=64-contraction matmuls measure ~7x slower per instruction than
    K=128 on this HW, so the logits matmul pads the moving operand:
    per-head qTz tiles keep the sibling head's 64 partition rows at
    zero, letting the full two-head kT tile serve as a K=128
    stationary.
  - AV uses attnT as stationary ([k,q] tiles from exp) and the
    65-wide vaug (V plus a ones column) as moving, accumulating
    av[q, d+1] per (head, q-tile).  The ones column yields the
    softmax denominator per q ON PARTITIONS, so the division is one
    DVE reciprocal + tensor_scalar_mul (broadcast-free).  The
    alternative (out [d+1, q], PE-broadcast of the recip row)
    measured ~90us slower end-to-end from serialized chains.
  - Output projection: PE-transpose ao[q,c] chunks, then wp matmuls;
    output written [q, co] directly.
"""

import numpy as np

import concourse.bass as bass
import concourse.tile as tile
from concourse import bacc, mybir
from concourse.bass_utils import run_bass_kernel_spmd
from concourse.masks import make_identity

B = 4
N = 2048          # key/value sequence length
C = 512           # model dim
H = 8
D = C // H        # 64
NCORES = 8
QSH = N // 2      # query rows per core (1024)

F32 = mybir.dt.float32
F32R = mybir.dt.float32r
BF16 = mybir.dt.bfloat16

P = 128
CT = C // P       # 4 tiles along any model-dim axis
KT = N // P       # 16 key tiles
QT = QSH // P     # 8 query tiles
QB = QSH // 512   # 2 query 512-blocks
NB = N // 512     # 4 key 512-blocks
DHT = CT          # 4 dh tiles (2 heads each)


def build_nc(with_bias: bool, reps: int = 1):
    nc = bacc.Bacc("TRN2", target_bir_lowering=False, debug=False)

    xT = nc.dram_tensor("xT", [C, QSH], BF16, kind="ExternalInput")
    mlpT = nc.dram_tensor("mlpT", [C, N], BF16, kind="ExternalInput")
    wqT = nc.dram_tensor("wqT", [C, C], BF16, kind="ExternalInput")
    wkT = nc.dram_tensor("wkT", [C, C], BF16, kind="ExternalInput")
    wvT = nc.dram_tensor("wvT", [C, C], BF16, kind="ExternalInput")
    wpT = nc.dram_tensor("wpT", [C, C], BF16, kind="ExternalInput")
    if with_bias:
        bqc = nc.dram_tensor("bqc", [C, 1], F32, kind="ExternalInput")
        bkc = nc.dram_tensor("bkc", [C, 1], F32, kind="ExternalInput")
        bvr = nc.dram_tensor("bvr", [1, C], F32, kind="ExternalInput")
        bpr = nc.dram_tensor("bpr", [1, C], F32, kind="ExternalInput")
    out = nc.dram_tensor("out", [QSH, C], F32, kind="ExternalOutput")

    with tile.TileContext(nc) as tc:
        from contextlib import ExitStack

        with ExitStack() as ctx:
            const = ctx.enter_context(tc.tile_pool(name="const", bufs=1))
            w_pool = ctx.enter_context(tc.tile_pool(name="w", bufs=1))
            x_pool = ctx.enter_context(tc.tile_pool(name="x", bufs=1))
            m_pool = ctx.enter_context(tc.tile_pool(name="mlp", bufs=1))
            qt_pool = ctx.enter_context(tc.tile_pool(name="qT", bufs=1))
            kt_pool = ctx.enter_context(tc.tile_pool(name="kT", bufs=1))
            v_pool = ctx.enter_context(tc.tile_pool(name="vaug", bufs=1))
            attn_pool = ctx.enter_context(tc.tile_pool(name="attnT", bufs=44))
            ao_pool = ctx.enter_context(tc.tile_pool(name="aoQ", bufs=1))
            aoTq_pool = ctx.enter_context(tc.tile_pool(name="aoTq", bufs=2))
            outst = ctx.enter_context(tc.tile_pool(name="outst", bufs=2))
            small = ctx.enter_context(tc.tile_pool(name="small", bufs=4))
            lp_ps = ctx.enter_context(
                tc.tile_pool(name="lp_ps", bufs=3, space="PSUM")
            )
            av_ps = ctx.enter_context(
                tc.tile_pool(name="av_ps", bufs=3, space="PSUM")
            )
            proj_ps = ctx.enter_context(
                tc.tile_pool(name="proj_ps", bufs=2, space="PSUM")
            )

            ident = const.tile([P, P], F32)
            make_identity(nc, ident)
            ident_bf = const.tile([P, P], BF16)
            nc.vector.tensor_copy(ident_bf[:], ident[:])
            ones_f = const.tile([1, P], F32)
            nc.vector.memset(ones_f[:], 1.0)
            ones_r = const.tile([1, P], F32R)
            nc.vector.tensor_copy(ones_r[:], ones_f[:])

            for _rep in range(reps):

                # ---- resident tiles ----
                wq = [w_pool.tile([P, C], BF16, tag=f"wq{i}", name=f"wq{i}")
                      for i in range(CT)]
                wk = [w_pool.tile([P, C], BF16, tag=f"wk{i}", name=f"wk{i}")
                      for i in range(CT)]
                wv = [w_pool.tile([P, C], BF16, tag=f"wv{i}", name=f"wv{i}")
                      for i in range(CT)]
                wp = [w_pool.tile([P, C], BF16, tag=f"wp{i}", name=f"wp{i}")
                      for i in range(CT)]
                xt = [x_pool.tile([P, QSH], BF16, tag=f"x{i}", name=f"x{i}")
                      for i in range(CT)]
                mt = [m_pool.tile([P, N], BF16, tag=f"m{i}", name=f"m{i}")
                      for i in range(CT)]
                # per-head query tiles, sibling head's rows zeroed (K=128
                # logits trick)
                qTz = [qt_pool.tile([P, QSH], BF16, tag=f"qz{h}", name=f"qz{h}")
                       for h in range(H)]
                kT = [kt_pool.tile([P, N], BF16, tag=f"kT{i}", name=f"kT{i}")
                      for i in range(DHT)]
                vaug = [v_pool.tile([P, H, D + 1], BF16, tag=f"v{i}",
                                    name=f"v{i}") for i in range(KT)]
                aoQ = [ao_pool.tile([P, C], BF16, tag=f"ao{i}", name=f"ao{i}")
                       for i in range(QT)]

                # ---- DMA prologue, critical-path first ----
                def dma_w(tiles, dram):
                    for i, t in enumerate(tiles):
                        nc.sync.dma_start(out=t[:], in_=dram[i * P:(i + 1) * P, :])

                def dma_chunk(t, dram, row0, col0, cols):
                    nc.sync.dma_start(
                        out=t[:, col0:col0 + cols],
                        in_=dram[row0:row0 + P, col0:col0 + cols],
                    )

                dma_w(wq, wqT)
                for i in range(CT):          # x qb0 chunks
                    dma_chunk(xt[i], xT, i * P, 0, 512)
                dma_w(wk, wkT)
                for i in range(CT):          # mlp nb0 chunks
                    dma_chunk(mt[i], mlpT, i * P, 0, 512)
                for i in range(CT):          # x qb1
                    dma_chunk(xt[i], xT, i * P, 512, 512)
                for nb in range(1, NB):      # rest of mlp
                    for i in range(CT):
                        dma_chunk(mt[i], mlpT, i * P, nb * 512, 512)
                dma_w(wv, wvT)
                dma_w(wp, wpT)

                if with_bias:
                    bq_col = [small.tile([P, 1], F32, tag=f"bq{i}", name=f"bq{i}")
                              for i in range(DHT)]
                    bk_col = [small.tile([P, 1], F32, tag=f"bk{i}", name=f"bk{i}")
                              for i in range(DHT)]
                    for i in range(DHT):
                        nc.sync.dma_start(out=bq_col[i][:],
                                          in_=bqc[i * P:(i + 1) * P, :])
                        nc.sync.dma_start(out=bk_col[i][:],
                                          in_=bkc[i * P:(i + 1) * P, :])
                    def load_row_r(dram_row, nm):
                        f = const.tile([1, C], F32, name=f"{nm}_f")
                        nc.sync.dma_start(out=f[:], in_=dram_row[:])
                        r = const.tile([1, C], F32R, name=f"{nm}_r")
                        nc.vector.tensor_copy(r[:], f[:])
                        return r
                    bv_r = load_row_r(bvr, "bv")
                    bp_r = load_row_r(bpr, "bp")

                # ---- work chunks ----
                def _proj_mm(wt, dh, src, nb_):
                    ps = proj_ps.tile([P, 512], F32, tag="proj", name="ps_qk")
                    for cb in range(CT):
                        nc.tensor.matmul(
                            ps[:],
                            wt[cb][:, dh * P:(dh + 1) * P],
                            src[cb][:, nb_ * 512:(nb_ + 1) * 512],
                            start=(cb == 0),
                            stop=(cb == CT - 1),
                        )
                    return ps

                def proj_q(dh, qb):
                    # evict each head's 64 rows into its zero-padded tile
                    ps = _proj_mm(wq, dh, xt, qb)
                    for par in range(2):
                        h = 2 * dh + par
                        po = par * D
                        sl = qTz[h][po:po + D, qb * 512:(qb + 1) * 512]
                        if with_bias:
                            nc.vector.tensor_scalar_add(
                                sl, ps[po:po + D, :],
                                bq_col[dh][po:po + D, :])
                        else:
                            nc.vector.tensor_copy(sl, ps[po:po + D, :])

                def proj_k(dh, nb_):
                    ps = _proj_mm(wk, dh, mt, nb_)
                    sl = kT[dh][:, nb_ * 512:(nb_ + 1) * 512]
                    if with_bias:
                        nc.vector.tensor_scalar_add(sl, ps[:], bk_col[dh][:])
                    else:
                        nc.vector.tensor_copy(sl, ps[:])

                def proj_v(nt):
                    nb_ = nt // 4
                    ps = proj_ps.tile([P, 512], F32, tag="proj", name="ps_v")
                    for cb in range(CT):
                        nc.tensor.matmul(
                            ps[:],
                            mt[cb][:, nt * P:(nt + 1) * P],
                            wv[cb][:],
                            start=(cb == 0),
                            stop=(cb == CT - 1 and not with_bias),
                        )
                    if with_bias:
                        nc.tensor.matmul(ps[:], ones_r[:], bv_r[:],
                                         start=False, stop=True)
                    nc.vector.tensor_copy(
                        vaug[nt][:, :, 0:D],
                        ps[:].rearrange("p (h d) -> p h d", h=H),
                    )
                    nc.vector.memset(vaug[nt][:, :, D:D + 1], 1.0)

                attnT = {h: [] for h in range(H)}

                def logits_exp(h, kt):
                    dh = h // 2
                    at = attn_pool.tile([P, QSH], BF16, tag="attnT", name="at")
                    for qb in range(QB):
                        lp = lp_ps.tile([P, 512], F32, tag="lp", name="lp")
                        nc.tensor.matmul(
                            lp[:],
                            kT[dh][:, kt * P:(kt + 1) * P],
                            qTz[h][:, qb * 512:(qb + 1) * 512],
                            start=True,
                            stop=True,
                        )
                        nc.scalar.activation(
                            out=at[:, qb * 512:(qb + 1) * 512], in_=lp[:],
                            func=mybir.ActivationFunctionType.Exp,
                        )
                    attnT[h].append(at)

                def av_group(h, qt):
                    # av[q, d+1] for one (head, q-tile); the ones column of
                    # vaug gives the softmax denominator on partitions.
                    av = av_ps.tile([P, D + 1], F32, tag="av", name="av")
                    tiles = attnT[h]
                    for kt in range(KT):
                        nc.tensor.matmul(
                            av[:],
                            tiles[kt][:, qt * P:(qt + 1) * P],
                            vaug[kt][:, h, :],
                            start=(kt == 0),
                            stop=(kt == KT - 1),
                        )
                    recip = small.tile([P, 1], F32, tag="recip", name="recip")
                    nc.vector.reciprocal(recip[:], av[:, D:D + 1])
                    nc.vector.tensor_scalar_mul(
                        aoQ[qt][:, h * D:(h + 1) * D], av[:, 0:D], recip[:])

                def tail_qt(qt):
                    # transpose ao[q, c] chunks, then project: out[q, co]
                    ps_t = proj_ps.tile([P, 512], BF16, tag="proj", name="ps_t")
                    for mt_ in range(CT):
                        nc.tensor.transpose(
                            ps_t[:, mt_ * P:(mt_ + 1) * P],
                            aoQ[qt][:, mt_ * P:(mt_ + 1) * P],
                            ident_bf[:],
                        )
                    aoTq = aoTq_pool.tile([P, CT, P], BF16, tag="aoTq",
                                          name="aoTq")
                    nc.vector.tensor_copy(
                        aoTq[:], ps_t[:].rearrange("p (m q) -> p m q", m=CT))
                    po = proj_ps.tile([P, 512], F32, tag="proj", name="ps_o")
                    for mt_ in range(CT):
                        nc.tensor.matmul(
                            po[:],
                            aoTq[:, mt_, :],
                            wp[mt_][:],
                            start=(mt_ == 0),
                            stop=(mt_ == CT - 1 and not with_bias),
                        )
                    if with_bias:
                        nc.tensor.matmul(po[:], ones_r[:], bp_r[:],
                                         start=False, stop=True)
                    o = outst.tile([P, C], F32, tag="outst", name="outst")
                    nc.vector.tensor_copy(o[:], po[:])
                    nc.sync.dma_start(out=out[qt * P:(qt + 1) * P, :], in_=o[:])

                # ---- schedule ----
                def Qp(dh, qb):
                    proj_q(dh, qb)

                def Kp(dh, nb):
                    proj_k(dh, nb)

                # zero the sibling-head rows of the padded query tiles
                for h in range(H):
                    zo = (1 - h % 2) * D
                    nc.vector.memset(qTz[h][zo:zo + D, :], 0.0)

                # prologue: qT/kT for heads 0/1, logits+exp head 0
                for qb in range(QB):
                    Qp(0, qb)
                for nb in range(NB):
                    Kp(0, nb)
                    for kt in range(4 * nb, 4 * nb + 4):
                        logits_exp(0, kt)

                # Slot s feeds logits for head s+1, runs AV groups of lagged
                # heads, and drains deadline-ordered projection chunks.
                # All proj_v chunks must be emitted before the first av_group
                # (slot 2): each av_group reads every vaug tile.
                extra = {
                    0: [lambda qb=qb: Qp(1, qb) for qb in range(QB)]
                    + [lambda nb=nb: Kp(1, nb) for nb in range(NB)]
                    + [lambda nt=nt: proj_v(nt) for nt in range(0, 4)],
                    1: [lambda nt=nt: proj_v(nt) for nt in range(4, 16)]
                    + [lambda qb=qb: Qp(2, qb) for qb in range(QB)]
                    + [lambda nb=nb: Kp(2, nb) for nb in range(NB)],
                    2: [lambda qb=qb: Qp(3, qb) for qb in range(QB)]
                    + [lambda nb=nb: Kp(3, nb) for nb in range(NB)],
                    3: [], 4: [], 5: [], 6: [], 7: [],
                }
                # av groups (head, q-tile) per slot; av(h) runs in slot h+2
                av_sched = {
                    0: [], 1: [],
                    2: [(0, qt) for qt in range(QT)],
                    3: [(1, qt) for qt in range(QT)],
                    4: [(2, qt) for qt in range(QT)],
                    5: [(3, qt) for qt in range(QT)],
                    6: [(4, qt) for qt in range(QT)]
                    + [(5, qt) for qt in range(QT // 2)],
                    7: [(5, qt) for qt in range(QT // 2, QT)]
                    + [(6, qt) for qt in range(QT)]
                    + [(7, qt) for qt in range(QT // 2)],
                }

                for slot in range(H):
                    nxt = slot + 1
                    work = list(extra[slot])
                    avq = list(av_sched[slot])
                    per_i = (len(avq) + 7) // 8
                    for i in range(8):
                        if nxt < H:
                            logits_exp(nxt, 2 * i)
                            logits_exp(nxt, 2 * i + 1)
                        if work:
                            work.pop(0)()
                        for _ in range(per_i):
                            if avq:
                                h_, qt_ = avq.pop(0)
                                av_group(h_, qt_)
                    while work:
                        work.pop(0)()
                    for h_, qt_ in avq:
                        av_group(h_, qt_)

                # tail: finish av(7), then per-qtile output projection
                for qt in range(QT // 2, QT):
                    av_group(7, qt)
                for qt in range(QT):
                    tail_qt(qt)

    nc.compile()
    return nc


_CACHE: dict = {}


def get_nc(with_bias: bool):
    key = ("nc", with_bias)
    if key not in _CACHE:
        _CACHE[key] = build_nc(with_bias)
    return _CACHE[key]


def _bf16(a):
    import ml_dtypes
    return np.ascontiguousarray(a.astype(ml_dtypes.bfloat16))


def make_in_maps(inputs: dict) -> tuple[list[dict], bool]:
    x = np.asarray(inputs["x"], dtype=np.float32)
    mlp = np.asarray(inputs["mlp_out"], dtype=np.float32)
    Wq = np.asarray(inputs["Wq"], dtype=np.float32)
    Wk = np.asarray(inputs["Wk"], dtype=np.float32)
    Wv = np.asarray(inputs["Wv"], dtype=np.float32)
    Wp = np.asarray(inputs["Wp"], dtype=np.float32)
    bq = np.asarray(inputs["bq"], dtype=np.float32)
    bk = np.asarray(inputs["bk"], dtype=np.float32)
    bv = np.asarray(inputs["bv"], dtype=np.float32)
    bp = np.asarray(inputs["bp"], dtype=np.float32)

    with_bias = bool(np.any(bq) or np.any(bk) or np.any(bv) or np.any(bp))

    wqT = _bf16(Wq.T)
    wkT = _bf16(Wk.T)
    wvT = _bf16(Wv.T)
    wpT = _bf16(Wp.T)

    in_maps = []
    for c in range(NCORES):
        b, half = c // 2, c % 2
        xs = _bf16(x[b, half * QSH:(half + 1) * QSH, :].T)
        ms = _bf16(mlp[b].T)
        m = {
            "xT": xs, "mlpT": ms,
            "wqT": wqT, "wkT": wkT, "wvT": wvT, "wpT": wpT,
        }
        if with_bias:
            m["bqc"] = bq.reshape(C, 1)
            m["bkc"] = bk.reshape(C, 1)
            m["bvr"] = bv.reshape(1, C)
            m["bpr"] = bp.reshape(1, C)
        in_maps.append(m)
    return in_maps, with_bias


def kernel(**inputs) -> np.ndarray:
    in_maps, with_bias = make_in_maps(inputs)
    nc = get_nc(with_bias)
    res = run_bass_kernel_spmd(nc, in_maps, list(range(NCORES)))
    full = np.empty((B, N, C), dtype=np.float32)
    for c in range(NCORES):
        b, half = c // 2, c % 2
        full[b, half * QSH:(half + 1) * QSH, :] = res.results[c]["out"]
    return full
